# revision 3
# baseline (speedup 1.0000x reference)
"""Trainium2 Bass kernel for nn_EnsembleModel (ensemble recommender).

Strategy (v2):
- All big matmuls in fp16 3-term split precision: x ~ xh + xl (fp16 planes,
  inputs pre-scaled by 2^10 so residuals stay normal-range), and
  x@y ~ xh@yh + xh@yl + xl@yh accumulated in fp32 PSUM. This runs at
  ~1 cyc/row vs fp32's ~4.9 cyc/row on the PE (net ~1.5x PE win) and gives
  ~8/61440 index mismatches in simulation (plain fp32 gives 4-6).
- Item-sharded contractions + on-device collectives (8 cores):
  each core holds a contiguous 2500-item slice of X^T / UnT / W-prior
  planes, computes partial hid [1024,512] and partial simT [2048,1024] for
  the FULL batch, then ReduceScatter(hid) -> own 128 batch rows, and
  2x AllReduce(simT halves). k_full is computed item-sharded for the full
  batch and redistributed to batch-sharding with 3x AllToAll.
  Per-core HBM traffic drops ~381MB -> ~140MB, hidden under PE.
- Top-k via DVE max/max_index/match_replace (top-8 per pass) with the
  one-hot value-match + min-reduce index-extraction trick (OFF offset),
  index translation through the scatter maps via indirect DMA gathers.
"""
import sys

if "/opt/trn_rl_repo" not in sys.path:
    sys.path.insert(0, "/opt/trn_rl_repo")

import numpy as np
from contextlib import ExitStack

import concourse.bass as bass
import concourse.bacc as bacc
import concourse.mybir as mybir
from concourse.tile import TileContext
from concourse.bass_utils import run_bass_kernel_spmd

P = 128
B, N, N_TOP, N_MID, D, N_USERS, K = 1024, 20000, 2000, 8000, 256, 2000, 20
N_CORES = 8
B_LOC = B // N_CORES           # 128
SH = N // N_CORES              # 2500 items per core shard
KT = 20                        # k-tiles per shard (2560 padded)
SH_PAD = KT * P                # 2560
NU = 2048                      # users padded
UT = NU // P                   # 16 user tiles
D2 = 2 * D                     # 512 stacked hid dims (top|mid)
BT = B // P                    # 8 batch tiles
CH = 500                       # top-k chunk width
KF_GROUPS = ([0, 1], [2, 3], [4])   # kfull item-chunk groups per shard
WMAX = 2 * CH                  # widest group
NCH_MID = N_MID // CH          # 16
NCH_TOP = N_TOP // CH          # 4
C_MID = NCH_MID * 8            # 128 candidate cols
NCH_CF = N // CH               # 40 chunks total over items
C_CF = NCH_CF * 8              # 320 candidate cols
NEG = -1e30
OFF = 1.0e6
SC = 1024.0                    # 2^10 pre-scale for fp16 splits

F32 = mybir.dt.float32
F16 = mybir.dt.float16
U32 = mybir.dt.uint32
I32 = mybir.dt.int32


def build_program():
    nc = bacc.Bacc(None, target_bir_lowering=False, num_devices=N_CORES)

    # per-core inputs (item-shard slices; fp16 hi/lo planes, pre-scaled x2^10)
    xh_d = nc.dram_tensor("xh", [SH_PAD, B], F16, kind="ExternalInput")
    xl_d = nc.dram_tensor("xl", [SH_PAD, B], F16, kind="ExternalInput")
    uh_d = nc.dram_tensor("uh", [SH_PAD, NU], F16, kind="ExternalInput")
    ul_d = nc.dram_tensor("ul", [SH_PAD, NU], F16, kind="ExternalInput")
    wh_d = nc.dram_tensor("wh", [SH_PAD, D2], F16, kind="ExternalInput")
    wl_d = nc.dram_tensor("wl", [SH_PAD, D2], F16, kind="ExternalInput")
    wsdh_d = nc.dram_tensor("wsdh", [D, N_TOP], F16, kind="ExternalInput")
    wsdl_d = nc.dram_tensor("wsdl", [D, N_TOP], F16, kind="ExternalInput")
    wmdh_d = nc.dram_tensor("wmdh", [D, N_MID], F16, kind="ExternalInput")
    wmdl_d = nc.dram_tensor("wmdl", [D, N_MID], F16, kind="ExternalInput")
    rh_d = nc.dram_tensor("rh", [NU, SH], F16, kind="ExternalInput")
    rl_d = nc.dram_tensor("rl", [NU, SH], F16, kind="ExternalInput")
    tmap = nc.dram_tensor("tmap", [N_TOP, 1], I32, kind="ExternalInput")
    mmap = nc.dram_tensor("mmap", [N_MID, 1], I32, kind="ExternalInput")
    ident_d = nc.dram_tensor("ident", [P, P], F32, kind="ExternalInput")
    cb_mid_d = nc.dram_tensor("cb_mid", [P, C_MID], F32, kind="ExternalInput")
    cb_cf_d = nc.dram_tensor("cb_cf", [P, C_CF], F32, kind="ExternalInput")
    out_d = nc.dram_tensor("out", [P, 3, K], I32, kind="ExternalOutput")

    RG = [list(range(N_CORES))]

    with TileContext(nc) as tc, ExitStack() as ctx:
        dram = ctx.enter_context(tc.tile_pool(name="dram", bufs=1, space="DRAM"))
        # collective bounce buffers
        hid_in = dram.tile([B, D2], F32, name="hid_in")
        hid_rs = dram.tile([P, D2], F32, name="hid_rs")
        sim_in_a = dram.tile([NU // 2, B], F32, name="sim_in_a")
        sim_in_b = dram.tile([NU // 2, B], F32, name="sim_in_b")
        sim_ar_a = dram.tile([NU // 2, B], F32, name="sim_ar_a", addr_space="Shared")
        sim_ar_b = dram.tile([NU // 2, B], F32, name="sim_ar_b", addr_space="Shared")
        kf_ins, kf_outs = [], []
        for gi, chs in enumerate(KF_GROUPS):
            w = len(chs) * CH
            kf_ins.append(dram.tile([B, w], F32, name=f"kf_in{gi}"))
            kf_outs.append(dram.tile([B, w], F32, name=f"kf_out{gi}"))

        cpool = ctx.enter_context(tc.tile_pool(name="cpool", bufs=1))
        ident = cpool.tile([P, P], F32, tag="ident")
        nc.sync.dma_start(out=ident[:], in_=ident_d[:, :])
        cb_mid = cpool.tile([P, C_MID], F32, tag="cbm")
        nc.sync.dma_start(out=cb_mid[:], in_=cb_mid_d[:, :])
        cb_cf = cpool.tile([P, C_CF], F32, tag="cbc")
        nc.sync.dma_start(out=cb_cf[:], in_=cb_cf_d[:, :])

        pp = ctx.enter_context(tc.tile_pool(name="pp", bufs=8, space="PSUM"))
        scr = ctx.enter_context(tc.tile_pool(name="scr", bufs=2))
        opool = ctx.enter_context(tc.tile_pool(name="opool", bufs=1))

        # =========== Phase 1: priors partial + simT partial (item shard) ====
        with tc.tile_pool(name="xtp", bufs=1) as xtp, \
             tc.tile_pool(name="ustream", bufs=4) as ustream, \
             tc.tile_pool(name="stg1", bufs=4) as stg1:
            # resident X^T planes: [128, KT*B], column-block kt = item rows
            xh_sb = xtp.tile([P, KT * B], F16, tag="xh")
            xl_sb = xtp.tile([P, KT * B], F16, tag="xl")
            wh_sb = xtp.tile([P, KT * D2], F16, tag="wh")
            wl_sb = xtp.tile([P, KT * D2], F16, tag="wl")
            for kt in range(KT):
                nc.sync.dma_start(out=xh_sb[:, kt * B:(kt + 1) * B],
                                  in_=xh_d[kt * P:(kt + 1) * P, :])
                nc.sync.dma_start(out=xl_sb[:, kt * B:(kt + 1) * B],
                                  in_=xl_d[kt * P:(kt + 1) * P, :])
                nc.sync.dma_start(out=wh_sb[:, kt * D2:(kt + 1) * D2],
                                  in_=wh_d[kt * P:(kt + 1) * P, :])
                nc.sync.dma_start(out=wl_sb[:, kt * D2:(kt + 1) * D2],
                                  in_=wl_d[kt * P:(kt + 1) * P, :])

            # ---- priors: out hid [1024b, 512h]; lhsT = X^T tile, rhs = W ----
            for bt in range(BT):
                ps = pp.tile([P, D2], F32, tag="pp1", name=f"pp_pri{bt}")
                for kt in range(KT):
                    xh_t = xh_sb[:, kt * B + bt * P: kt * B + (bt + 1) * P]
                    xl_t = xl_sb[:, kt * B + bt * P: kt * B + (bt + 1) * P]
                    w_h = wh_sb[:, kt * D2:(kt + 1) * D2]
                    w_l = wl_sb[:, kt * D2:(kt + 1) * D2]
                    nc.tensor.matmul(ps[:, :], lhsT=xh_t, rhs=w_h,
                                     start=(kt == 0), stop=False)
                    nc.tensor.matmul(ps[:, :], lhsT=xh_t, rhs=w_l,
                                     start=False, stop=False)
                    nc.tensor.matmul(ps[:, :], lhsT=xl_t, rhs=w_h,
                                     start=False, stop=(kt == KT - 1))
                hst = stg1.tile([P, D2], F32, tag="hst", name=f"hst{bt}")
                nc.vector.tensor_copy(hst[:], ps[:])
                nc.sync.dma_start(out=hid_in[bt * P:(bt + 1) * P, :], in_=hst[:])
            nc.gpsimd.collective_compute(
                "ReduceScatter", mybir.AluOpType.add, replica_groups=RG,
                ins=[hid_in.opt()], outs=[hid_rs.opt()])

            # ---- simT: out [2048u, 1024b]; lhsT = UnT tile, rhs = X^T ------
            for g in range(4):                      # user groups of 512
                pss = {}
                for ut in range(4):
                    for bh in range(2):
                        pss[(ut, bh)] = pp.tile([P, 512], F32, tag="pp1",
                                                name=f"pp_sim{g}_{ut}_{bh}")
                for kt in range(KT):
                    uht = ustream.tile([P, 512], F16, tag="uht", name=f"uh{g}_{kt}")
                    nc.sync.dma_start(
                        out=uht[:],
                        in_=uh_d[kt * P:(kt + 1) * P, g * 512:(g + 1) * 512])
                    ult = ustream.tile([P, 512], F16, tag="ult", name=f"ul{g}_{kt}")
                    nc.sync.dma_start(
                        out=ult[:],
                        in_=ul_d[kt * P:(kt + 1) * P, g * 512:(g + 1) * 512])
                    for ut in range(4):
                        lh = uht[:, ut * P:(ut + 1) * P]
                        ll = ult[:, ut * P:(ut + 1) * P]
                        for bh in range(2):
                            ps = pss[(ut, bh)]
                            xh_r = xh_sb[:, kt * B + bh * 512: kt * B + (bh + 1) * 512]
                            xl_r = xl_sb[:, kt * B + bh * 512: kt * B + (bh + 1) * 512]
                            nc.tensor.matmul(ps[:, :], lhsT=lh, rhs=xh_r,
                                             start=(kt == 0), stop=False)
                            nc.tensor.matmul(ps[:, :], lhsT=lh, rhs=xl_r,
                                             start=False, stop=False)
                            nc.tensor.matmul(ps[:, :], lhsT=ll, rhs=xh_r,
                                             start=False, stop=(kt == KT - 1))
                # drain group g: 4 ut x 2 bh banks -> sim_in
                sim_in_t = sim_in_a if g < 2 else sim_in_b
                goff = (g % 2) * 512
                for ut in range(4):
                    for bh in range(2):
                        sst = stg1.tile([P, 512], F32, tag="sst",
                                        name=f"sst{g}_{ut}_{bh}")
                        nc.vector.tensor_copy(sst[:], pss[(ut, bh)][:])
                        nc.sync.dma_start(
                            out=sim_in_t[goff + ut * P: goff + (ut + 1) * P,
                                         bh * 512:(bh + 1) * 512],
                            in_=sst[:])
                if g == 1:
                    nc.gpsimd.collective_compute(
                        "AllReduce", mybir.AluOpType.add, replica_groups=RG,
                        ins=[sim_in_a.opt()], outs=[sim_ar_a.opt()])
                if g == 3:
                    nc.gpsimd.collective_compute(
                        "AllReduce", mybir.AluOpType.add, replica_groups=RG,
                        ins=[sim_in_b.opt()], outs=[sim_ar_b.opt()])

        # =========== Phase 2: decoders + top/mid top-k (own batch) =========
        def l2_extract(cand_vals, cand_idx_u, cb_tile, C, out_name):
            gidx = opool.tile([P, C], F32, tag=f"gidx{out_name}")
            nc.vector.tensor_copy(gidx[:], cand_idx_u[:])
            nc.vector.tensor_tensor(out=gidx[:], in0=gidx[:], in1=cb_tile[:],
                                    op=mybir.AluOpType.add)
            work = opool.tile([P, C], F32, tag=f"work{out_name}")
            nc.vector.tensor_copy(work[:], cand_vals[:])
            pidx = opool.tile([P, K], F32, tag=f"pidx{out_name}")
            for r in range(3):
                v8 = scr.tile([P, 8], F32, tag="v8l2")
                nc.vector.max(out=v8[:], in_=work[:])
                njj = 8 if r < 2 else K - 16
                for jj in range(njj):
                    j = r * 8 + jj
                    eqm = scr.tile([P, C], F32, tag=f"eq{out_name}")
                    nc.vector.tensor_tensor(out=eqm[:], in0=cand_vals[:],
                                            in1=v8[:, jj:jj + 1].to_broadcast([P, C]),
                                            op=mybir.AluOpType.is_equal)
                    nc.vector.tensor_tensor(out=eqm[:], in0=eqm[:], in1=gidx[:],
                                            op=mybir.AluOpType.mult)
                    nc.vector.tensor_reduce(out=pidx[:, j:j + 1], in_=eqm[:],
                                            axis=mybir.AxisListType.X,
                                            op=mybir.AluOpType.min)
                if r < 2:
                    nc.vector.match_replace(out=work[:], in_to_replace=v8[:],
                                            in_values=work[:], imm_value=NEG)
            nc.vector.tensor_scalar_add(pidx[:], pidx[:], OFF)
            return pidx

        with tc.tile_pool(name="decp", bufs=1) as decp, \
             tc.tile_pool(name="dstream", bufs=4) as dstream:
            # own-batch hid [128, 512] fp32 (scale 2^20) <- ReduceScatter out
            hid_own = decp.tile([P, D2], F32, tag="hid_own")
            nc.sync.dma_start(out=hid_own[:], in_=hid_rs[:, :])
            nc.vector.tensor_scalar_mul(hid_own[:], hid_own[:], 1.0 / SC)
            # transpose 4 tiles -> hidT [512h, 128b]; split fp16 planes
            hidT_h = decp.tile([P, 4 * P], F16, tag="hidT_h")
            hidT_l = decp.tile([P, 4 * P], F16, tag="hidT_l")
            for ht in range(4):
                tp = pp.tile([P, 512], F32, tag="pp1", name=f"pp_tp{ht}")
                nc.tensor.transpose(out=tp[:, 0:P],
                                    in_=hid_own[:, ht * P:(ht + 1) * P],
                                    identity=ident[:])
                tpf = decp.tile([P, P], F32, tag=f"tpf{ht}")
                nc.vector.tensor_copy(tpf[:], tp[:, 0:P])
                nc.vector.tensor_copy(hidT_h[:, ht * P:(ht + 1) * P], tpf[:])
                tpb = decp.tile([P, P], F32, tag=f"tpb{ht}")
                nc.vector.tensor_copy(tpb[:], hidT_h[:, ht * P:(ht + 1) * P])
                nc.vector.tensor_tensor(out=tpf[:], in0=tpf[:], in1=tpb[:],
                                        op=mybir.AluOpType.subtract)
                nc.vector.tensor_copy(hidT_l[:, ht * P:(ht + 1) * P], tpf[:])

            # ---- top decoder: [128, 2000] via 4 chunk banks ----
            top_sb = decp.tile([P, N_TOP], F32, tag="topsb")
            for c in range(NCH_TOP):
                ps = pp.tile([P, 512], F32, tag="pp1", name=f"pp_top{c}")
                for dt in range(2):
                    hh = hidT_h[:, dt * P:(dt + 1) * P]
                    hl = hidT_l[:, dt * P:(dt + 1) * P]
                    w_h = dstream.tile([P, CH], F16, tag="wdh", name=f"wsdh{c}_{dt}")
                    nc.sync.dma_start(
                        out=w_h[:],
                        in_=wsdh_d[dt * P:(dt + 1) * P, c * CH:(c + 1) * CH])
                    w_l = dstream.tile([P, CH], F16, tag="wdl", name=f"wsdl{c}_{dt}")
                    nc.sync.dma_start(
                        out=w_l[:],
                        in_=wsdl_d[dt * P:(dt + 1) * P, c * CH:(c + 1) * CH])
                    nc.tensor.matmul(ps[:, 0:CH], lhsT=hh, rhs=w_h[:],
                                     start=(dt == 0), stop=False)
                    nc.tensor.matmul(ps[:, 0:CH], lhsT=hh, rhs=w_l[:],
                                     start=False, stop=False)
                    nc.tensor.matmul(ps[:, 0:CH], lhsT=hl, rhs=w_h[:],
                                     start=False, stop=(dt == 1))
                nc.vector.tensor_copy(top_sb[:, c * CH:(c + 1) * CH], ps[:, 0:CH])

            top_idx = decp.tile([P, 24], U32, tag="topidx")
            for r in range(3):
                tv8 = scr.tile([P, 8], F32, tag="v8")
                nc.vector.max(out=tv8[:], in_=top_sb[:])
                nc.vector.max_index(out=top_idx[:, r * 8:(r + 1) * 8],
                                    in_max=tv8[:], in_values=top_sb[:])
                if r < 2:
                    nc.vector.match_replace(out=top_sb[:], in_to_replace=tv8[:],
                                            in_values=top_sb[:], imm_value=NEG)
            top_out = opool.tile([P, K], I32, tag="topout")
            for j in range(K):
                nc.gpsimd.indirect_dma_start(
                    out=top_out[:, j:j + 1], out_offset=None, in_=tmap[:, :],
                    in_offset=bass.IndirectOffsetOnAxis(ap=top_idx[:, j:j + 1], axis=0))
            nc.sync.dma_start(out=out_d[:, 0, :], in_=top_out[:])

            # ---- mid decoder: 16 chunks, candidates per chunk ----
            cand_vals_m = opool.tile([P, C_MID], F32, tag="cvm")
            cand_idx_m = opool.tile([P, C_MID], U32, tag="cim")
            for c in range(NCH_MID):
                ps = pp.tile([P, 512], F32, tag="pp1", name=f"pp_mid{c}")
                for dt in range(2):
                    hh = hidT_h[:, (2 + dt) * P:(3 + dt) * P]
                    hl = hidT_l[:, (2 + dt) * P:(3 + dt) * P]
                    w_h = dstream.tile([P, CH], F16, tag="wdh", name=f"wmdh{c}_{dt}")
                    nc.sync.dma_start(
                        out=w_h[:],
                        in_=wmdh_d[dt * P:(dt + 1) * P, c * CH:(c + 1) * CH])
                    w_l = dstream.tile([P, CH], F16, tag="wdl", name=f"wmdl{c}_{dt}")
                    nc.sync.dma_start(
                        out=w_l[:],
                        in_=wmdl_d[dt * P:(dt + 1) * P, c * CH:(c + 1) * CH])
                    nc.tensor.matmul(ps[:, 0:CH], lhsT=hh, rhs=w_h[:],
                                     start=(dt == 0), stop=False)
                    nc.tensor.matmul(ps[:, 0:CH], lhsT=hh, rhs=w_l[:],
                                     start=False, stop=False)
                    nc.tensor.matmul(ps[:, 0:CH], lhsT=hl, rhs=w_h[:],
                                     start=False, stop=(dt == 1))
                nc.vector.max(out=cand_vals_m[:, c * 8:(c + 1) * 8], in_=ps[:, 0:CH])
                nc.vector.max_index(out=cand_idx_m[:, c * 8:(c + 1) * 8],
                                    in_max=cand_vals_m[:, c * 8:(c + 1) * 8],
                                    in_values=ps[:, 0:CH])
            pidx_m = l2_extract(cand_vals_m, cand_idx_m, cb_mid, C_MID, "m")
            pidx_m_u = opool.tile([P, K], U32, tag="pmu")
            nc.vector.tensor_copy(pidx_m_u[:], pidx_m[:])
            mid_out = opool.tile([P, K], I32, tag="midout")
            for j in range(K):
                nc.gpsimd.indirect_dma_start(
                    out=mid_out[:, j:j + 1], out_offset=None, in_=mmap[:, :],
                    in_offset=bass.IndirectOffsetOnAxis(ap=pidx_m_u[:, j:j + 1], axis=0))
            nc.sync.dma_start(out=out_d[:, 1, :], in_=mid_out[:])

        # =========== Phase 3: kfull (item shard, full batch) + A2A =========
        cand_vals_c = opool.tile([P, C_CF], F32, tag="cvc")
        cand_idx_c = opool.tile([P, C_CF], U32, tag="cic")
        with tc.tile_pool(name="simp", bufs=1) as simp, \
             tc.tile_pool(name="rp", bufs=1) as rp, \
             tc.tile_pool(name="stg3", bufs=2) as stg3:
            # read back AllReduced simT, scale down, split into fp16 planes
            sh_sb = simp.tile([P, UT * B], F16, tag="sh")
            sl_sb = simp.tile([P, UT * B], F16, tag="sl")
            for ut in range(UT):
                src = sim_ar_a if ut < 8 else sim_ar_b
                so = (ut % 8) * P
                for bh in range(2):
                    sf = stg3.tile([P, 512], F32, tag="sf", name=f"sf{ut}_{bh}")
                    nc.sync.dma_start(out=sf[:],
                                      in_=src[so:so + P, bh * 512:(bh + 1) * 512])
                    nc.vector.tensor_scalar_mul(sf[:], sf[:], 1.0 / SC)
                    col = ut * B + bh * 512
                    nc.vector.tensor_copy(sh_sb[:, col:col + 512], sf[:])
                    sb32 = stg3.tile([P, 512], F32, tag="sb32", name=f"sb32_{ut}_{bh}")
                    nc.vector.tensor_copy(sb32[:], sh_sb[:, col:col + 512])
                    nc.vector.tensor_tensor(out=sf[:], in0=sf[:], in1=sb32[:],
                                            op=mybir.AluOpType.subtract)
                    nc.vector.tensor_copy(sl_sb[:, col:col + 512], sf[:])

            for gi, chs in enumerate(KF_GROUPS):
                gch = len(chs)
                wgrp = gch * CH
                coff = chs[0] * CH
                kf_in, kf_out = kf_ins[gi], kf_outs[gi]
                # resident R planes for this group: per ut [128, wgrp]
                r_h = rp.tile([P, UT * WMAX], F16, tag="r_h", name=f"r_h{gi}")
                r_l = rp.tile([P, UT * WMAX], F16, tag="r_l", name=f"r_l{gi}")
                for ut in range(UT):
                    nc.sync.dma_start(out=r_h[:, ut * WMAX: ut * WMAX + wgrp],
                                      in_=rh_d[ut * P:(ut + 1) * P, coff:coff + wgrp])
                    nc.sync.dma_start(out=r_l[:, ut * WMAX: ut * WMAX + wgrp],
                                      in_=rl_d[ut * P:(ut + 1) * P, coff:coff + wgrp])
                for bt in range(BT):
                    pss = [pp.tile([P, 512], F32, tag="pp1",
                                   name=f"pp_kf{gi}_{bt}_{c}")
                           for c in range(gch)]
                    for ut in range(UT):
                        sh_t = sh_sb[:, ut * B + bt * P: ut * B + (bt + 1) * P]
                        sl_t = sl_sb[:, ut * B + bt * P: ut * B + (bt + 1) * P]
                        for c in range(gch):
                            rh_t = r_h[:, ut * WMAX + c * CH: ut * WMAX + (c + 1) * CH]
                            nc.tensor.matmul(pss[c][:, 0:CH], lhsT=sh_t, rhs=rh_t,
                                             start=(ut == 0), stop=False)
                        for c in range(gch):
                            rl_t = r_l[:, ut * WMAX + c * CH: ut * WMAX + (c + 1) * CH]
                            nc.tensor.matmul(pss[c][:, 0:CH], lhsT=sh_t, rhs=rl_t,
                                             start=False, stop=False)
                        for c in range(gch):
                            rh_t = r_h[:, ut * WMAX + c * CH: ut * WMAX + (c + 1) * CH]
                            nc.tensor.matmul(pss[c][:, 0:CH], lhsT=sl_t, rhs=rh_t,
                                             start=False, stop=(ut == UT - 1))
                    kst = stg3.tile([P, wgrp], F32, tag="kst", name=f"kst{gi}_{bt}")
                    for c in range(gch):
                        nc.vector.tensor_copy(kst[:, c * CH:(c + 1) * CH],
                                              pss[c][:, 0:CH])
                    nc.sync.dma_start(out=kf_in[bt * P:(bt + 1) * P, :], in_=kst[:])
                nc.gpsimd.collective_compute(
                    "AllToAll", mybir.AluOpType.bypass, replica_groups=RG,
                    ins=[kf_in.opt()], outs=[kf_out.opt()])

            # readback + candidates (slot order must match cb_cf host const)
            slot = 0
            for gi, chs in enumerate(KF_GROUPS):
                gch = len(chs)
                wgrp = gch * CH
                kf_out = kf_outs[gi]
                for j in range(BT):
                    kt_t = stg3.tile([P, wgrp], F32, tag="kfrb", name=f"kfrb{gi}_{j}")
                    nc.sync.dma_start(out=kt_t[:], in_=kf_out[j * P:(j + 1) * P, :])
                    for c in range(gch):
                        nc.vector.max(out=cand_vals_c[:, slot * 8:(slot + 1) * 8],
                                      in_=kt_t[:, c * CH:(c + 1) * CH])
                        nc.vector.max_index(
                            out=cand_idx_c[:, slot * 8:(slot + 1) * 8],
                            in_max=cand_vals_c[:, slot * 8:(slot + 1) * 8],
                            in_values=kt_t[:, c * CH:(c + 1) * CH])
                        slot += 1

        pidx_c = l2_extract(cand_vals_c, cand_idx_c, cb_cf, C_CF, "c")
        cf_out = opool.tile([P, K], I32, tag="cfout")
        nc.vector.tensor_copy(cf_out[:], pidx_c[:])
        nc.sync.dma_start(out=out_d[:, 2, :], in_=cf_out[:])

    nc.compile()
    return nc


_NC_CACHE = None


def _get_program():
    global _NC_CACHE
    if _NC_CACHE is None:
        _NC_CACHE = build_program()
    return _NC_CACHE


def _split16(a):
    hi = a.astype(np.float16)
    lo = (a - hi.astype(np.float32)).astype(np.float16)
    return hi, lo


def cf_slot_bases():
    bases = []
    for chs in KF_GROUPS:
        for j in range(BT):
            for c in chs:
                bases.append(j * SH + c * CH)
    return np.asarray(bases, np.float32)


def prepare_in_maps(X, user_ratings, top_map, mid_map, Wsp, Wmp, Wsd, Wmd):
    X = np.asarray(X, np.float32)
    R = np.asarray(user_ratings, np.float32)
    Wsp = np.asarray(Wsp, np.float32)
    Wmp = np.asarray(Wmp, np.float32)
    Wsd = np.asarray(Wsd, np.float32)
    Wmd = np.asarray(Wmd, np.float32)
    top_map = np.asarray(top_map, np.int32).reshape(N_TOP, 1)
    mid_map = np.asarray(mid_map, np.int32).reshape(N_MID, 1)
    S = np.float32(SC)

    # Un exactly as the reference computes it (fp32 elementwise)
    norms = np.linalg.norm(R, axis=1).astype(np.float32)
    Un = R / (norms[:, None] + np.float32(1e-8))

    XTfull = X.T * S                                    # [N, B]
    UTfull = Un.T * S                                   # [N, N_USERS]
    Wfull = np.concatenate([Wsp, Wmp], axis=1) * S      # [N, 512]
    xh = np.zeros((N_CORES, SH_PAD, B), np.float16)
    xl = np.zeros((N_CORES, SH_PAD, B), np.float16)
    uh = np.zeros((N_CORES, SH_PAD, NU), np.float16)
    ul = np.zeros((N_CORES, SH_PAD, NU), np.float16)
    wh = np.zeros((N_CORES, SH_PAD, D2), np.float16)
    wl = np.zeros((N_CORES, SH_PAD, D2), np.float16)
    for c in range(N_CORES):
        sl_ = slice(c * SH, (c + 1) * SH)
        xh[c, :SH], xl[c, :SH] = _split16(XTfull[sl_])
        uh[c, :SH, :N_USERS], ul[c, :SH, :N_USERS] = _split16(UTfull[sl_])
        wh[c, :SH], wl[c, :SH] = _split16(Wfull[sl_])

    wsdh, wsdl = _split16(Wsd * S)
    wmdh, wmdl = _split16(Wmd * S)
    rh = np.zeros((N_CORES, NU, SH), np.float16)
    rl = np.zeros((N_CORES, NU, SH), np.float16)
    Rs = R * S
    for c in range(N_CORES):
        rh[c, :N_USERS], rl[c, :N_USERS] = _split16(Rs[:, c * SH:(c + 1) * SH])

    ident = np.eye(P, dtype=np.float32)
    cb_mid = np.broadcast_to(
        (np.repeat(np.arange(NCH_MID, dtype=np.float32) * CH, 8) - np.float32(OFF)),
        (P, C_MID)).copy()
    cb_cf = np.broadcast_to(
        (np.repeat(cf_slot_bases(), 8) - np.float32(OFF)), (P, C_CF)).copy()

    in_maps = []
    for c in range(N_CORES):
        in_maps.append(dict(
            xh=np.ascontiguousarray(xh[c]), xl=np.ascontiguousarray(xl[c]),
            uh=np.ascontiguousarray(uh[c]), ul=np.ascontiguousarray(ul[c]),
            wh=np.ascontiguousarray(wh[c]), wl=np.ascontiguousarray(wl[c]),
            wsdh=wsdh, wsdl=wsdl, wmdh=wmdh, wmdl=wmdl,
            rh=np.ascontiguousarray(rh[c]), rl=np.ascontiguousarray(rl[c]),
            tmap=top_map, mmap=mid_map,
            ident=ident, cb_mid=cb_mid, cb_cf=cb_cf))
    return in_maps


def kernel(X, mask, top_map, mid_map, user_ratings, user_personalities,
           Wsp, bsp, Wsd, bsd, Wmp, bmp, Wmd, bmd, k, **_unused):
    assert int(k) == K
    in_maps = prepare_in_maps(X, user_ratings, top_map, mid_map, Wsp, Wmp, Wsd, Wmd)
    nc = _get_program()
    res = run_bass_kernel_spmd(nc, in_maps, core_ids=list(range(N_CORES)))
    out = np.concatenate([r["out"] for r in res.results], axis=0)
    return out.astype(np.int32)
